# revision 24
# baseline (speedup 1.0000x reference)
"""Binarized 3x3 conv (sign(W) conv + bias) on 8 Trainium2 NeuronCores.

Problem (hardcoded):
  x:      (32, 256, 56, 56) f32
  weight: (256, 256, 3, 3)  f32  -> sign-binarized
  bias:   (256,)            f32
  out:    (32, 256, 56, 56) f32  (stride 1, pad 1)

Sharding/marshaling (host): data-parallel over batch — 4 images per
core, weight/bias replicated. The weight is passed to the device
pre-permuted to [IC, OC, 3, 3] (a pure layout transform, like the batch
slicing) so that the matmul lhsT tiles come out of a single on-device
sign() with no transposes; all arithmetic (sign, conv, bias) runs on
device.

Per-core kernel: conv = sum over the 9 taps of shifted 1x1 convs. x is
cast to bf16 (DVE) into a zero-padded [128, 2, 58, 58] SBUF image in
row slabs so matmuls can start before the whole image has landed.
Weights are sign-binarized per (ic-chunk, oc-chunk) on ACT (Sign: 0->0,
matching jnp.sign) directly into tap-major lhsT tiles [ic, oc] (bf16 is
exact for ±1/0). Each output tile [oc=128, 8 rows x 56 cols = 448]
accumulates 2 ic-chunks x 9 taps = 18 bf16 matmuls in one PSUM bank
(fp32 accumulate), then gets the per-channel bias added on DVE and is
DMA'd out.
"""

import numpy as np

import concourse.bass as bass
import concourse.mybir as mybir
import concourse.tile as tile
from concourse import bacc
from concourse.bass_utils import run_bass_kernel_spmd

N_CORES = 8
B = 32
B_PER = B // N_CORES  # 4 images per core
IC = OC = 256
H = W = 56
K = 3
R = 8               # output rows per matmul group
G = H // R          # 7 row groups
NCH = IC // 128     # 2 ic chunks
OCH = OC // 128     # 2 oc chunks
SLAB = 14           # x load/cast row-slab height
NSLAB = H // SLAB   # 4 slabs

# Results of the last run_bass_kernel_spmd call (exec_time_ns etc.) for
# introspection by test harnesses; not used for grading.
LAST_RESULTS = None

_CACHED_NC = None


def _build_nc() -> bass.Bass:
    nc = bacc.Bacc()
    x = nc.dram_tensor("x", [B_PER, IC, H, W], mybir.dt.float32,
                       kind="ExternalInput")
    wt = nc.dram_tensor("weight_t", [IC, K, K, OC], mybir.dt.float32,
                        kind="ExternalInput")
    bs = nc.dram_tensor("bias", [OC], mybir.dt.float32, kind="ExternalInput")
    out = nc.dram_tensor("out", [B_PER, OC, H, W], mybir.dt.float32,
                         kind="ExternalOutput")

    with tile.TileContext(nc) as tc:
        with (
            tc.tile_pool(name="const", bufs=1) as const_pool,
            tc.tile_pool(name="wprep", bufs=2) as wprep_pool,
            tc.tile_pool(name="xs", bufs=6) as xs_pool,
            tc.tile_pool(name="xp", bufs=2) as xp_pool,
            tc.tile_pool(name="osb", bufs=4) as out_pool,
            tc.tile_pool(name="psum", bufs=2, space="PSUM") as psum_pool,
        ):
            # ---- weight prep: sign-binarize into lhsT tap tiles ----
            # w_taps[ic_part, c, o, t, oc] : lhsT for (ic chunk c, oc chunk o, tap t)
            w_taps = const_pool.tile([128, NCH, OCH, K * K, 128],
                                     mybir.dt.bfloat16, tag="wtaps")

            def prep_weights(o, split=1):
                for c in range(NCH):
                    # host-permuted weight [ic, ky, kx, oc] makes both the
                    # sign read and the lhsT write fully contiguous
                    wf = wprep_pool.tile([128, K * K, 128], mybir.dt.float32,
                                         tag="wf", bufs=2, name=f"wf_{o}_{c}")
                    wsrc = wt[c * 128:(c + 1) * 128, :, :,
                              o * 128:(o + 1) * 128].rearrange(
                                  "i a b o -> i (a b) o")
                    taps = (K * K) // split
                    for j in range(split):
                        nc.sync.dma_start(
                            wf[:, j * taps:(j + 1) * taps, :],
                            wsrc[:, j * taps:(j + 1) * taps, :])
                    nc.scalar.sign(w_taps[:, c, o, :, :], wf)

            prep_weights(0, split=3)

            def load_slab(n, xpd, s, split=1):
                for c in range(NCH):
                    xs = xs_pool.tile([128, SLAB, W], mybir.dt.float32,
                                      tag="xs", name=f"xs_{n}_{s}_{c}")
                    rows = SLAB // split
                    for j in range(split):
                        nc.sync.dma_start(
                            xs[:, j * rows:(j + 1) * rows, :],
                            x[n, c * 128:(c + 1) * 128,
                              s * SLAB + j * rows:s * SLAB + (j + 1) * rows,
                              :])
                    nc.vector.tensor_copy(
                        out=xpd[:, c, 1 + s * SLAB:1 + (s + 1) * SLAB,
                                1:W + 1],
                        in_=xs)

            # ---- per-image conv ----
            for n in range(B_PER):
                # zero-padded bf16 image [128, c, 58, 58]
                xpd = xp_pool.tile([128, NCH, H + 2, W + 2], mybir.dt.bfloat16,
                                   tag="xpd", name=f"xpd_{n}")
                for c in range(NCH):
                    nc.any.memset(xpd[:, c, 0, :], 0.0)
                    nc.any.memset(xpd[:, c, H + 1, :], 0.0)
                    nc.any.memset(xpd[:, c, 1:H + 1, 0], 0.0)
                    nc.any.memset(xpd[:, c, 1:H + 1, W + 1], 0.0)
                # load + cast in row slabs so matmuls can start early
                for s in range(NSLAB):
                    load_slab(n, xpd, s, split=2 if (n == 0 and s <= 1) else 1)
                    if n == 0 and s == 0:
                        bias_sb = const_pool.tile([128, OCH],
                                                  mybir.dt.float32, tag="bias")
                        nc.sync.dma_start(
                            bias_sb, bs.rearrange("(a p) -> p a", p=128))
                        prep_weights(1)

                for o in range(OCH):
                    for g in range(G):
                        ps = psum_pool.tile([128, R, W], mybir.dt.float32,
                                            tag="acc", bufs=8)
                        for c in range(NCH):
                            for ky in range(K):
                                for kx in range(K):
                                    t = ky * K + kx
                                    nc.tensor.matmul(
                                        ps,
                                        w_taps[:, c, o, t, :],
                                        xpd[:, c, g * R + ky:g * R + ky + R,
                                            kx:kx + W],
                                        start=(c == 0 and t == 0),
                                        stop=(c == NCH - 1 and t == K * K - 1),
                                    )
                        osb = out_pool.tile([128, R, W], mybir.dt.float32,
                                            tag="osb")
                        nc.vector.tensor_tensor(
                            osb, ps,
                            bias_sb[:, o:o + 1, None].to_broadcast((128, R, W)),
                            mybir.AluOpType.add)
                        nc.sync.dma_start(
                            out[n, o * 128:(o + 1) * 128,
                                g * R:(g + 1) * R, :],
                            osb)
    nc.finalize()
    return nc


def kernel(x: np.ndarray, weight: np.ndarray, bias: np.ndarray) -> np.ndarray:
    global LAST_RESULTS, _CACHED_NC
    assert x.shape == (B, IC, H, W)
    if _CACHED_NC is None:
        _CACHED_NC = _build_nc()
    nc = _CACHED_NC

    # pure layout transform: [OC, IC, 3, 3] -> [IC, 3, 3, OC]
    weight_t = np.ascontiguousarray(
        np.asarray(weight, dtype=np.float32).transpose(1, 2, 3, 0))
    bias = np.ascontiguousarray(bias, dtype=np.float32)
    in_maps = [
        {
            "x": np.ascontiguousarray(x[i * B_PER:(i + 1) * B_PER],
                                      dtype=np.float32),
            "weight_t": weight_t,
            "bias": bias,
        }
        for i in range(N_CORES)
    ]
    res = run_bass_kernel_spmd(nc, in_maps, core_ids=list(range(N_CORES)))
    LAST_RESULTS = res
    return np.concatenate([res.results[i]["out"] for i in range(N_CORES)],
                          axis=0)


# revision 27
# speedup vs baseline: 1.0113x; 1.0113x over previous
"""Binarized 3x3 conv (sign(W) conv + bias) on 8 Trainium2 NeuronCores.

Problem (hardcoded):
  x:      (32, 256, 56, 56) f32
  weight: (256, 256, 3, 3)  f32  -> sign-binarized
  bias:   (256,)            f32
  out:    (32, 256, 56, 56) f32  (stride 1, pad 1)

Sharding/marshaling (host): data-parallel over batch — 4 images per
core, weight/bias replicated. The weight is passed to the device
pre-permuted to [IC, OC, 3, 3] (a pure layout transform, like the batch
slicing) so that the matmul lhsT tiles come out of a single on-device
sign() with no transposes; all arithmetic (sign, conv, bias) runs on
device.

Per-core kernel: conv = sum over the 9 taps of shifted 1x1 convs. x is
cast to bf16 (DVE) into a zero-padded [128, 2, 58, 58] SBUF image in
row slabs so matmuls can start before the whole image has landed.
Weights are sign-binarized per (ic-chunk, oc-chunk) on ACT (Sign: 0->0,
matching jnp.sign) directly into tap-major lhsT tiles [ic, oc] (bf16 is
exact for ±1/0). Each output tile [oc=128, 8 rows x 56 cols = 448]
accumulates 2 ic-chunks x 9 taps = 18 bf16 matmuls in one PSUM bank
(fp32 accumulate), then gets the per-channel bias added on DVE and is
DMA'd out.
"""

import numpy as np

import concourse.bass as bass
import concourse.mybir as mybir
import concourse.tile as tile
from concourse import bacc
from concourse.bass_utils import run_bass_kernel_spmd

N_CORES = 8
B = 32
B_PER = B // N_CORES  # 4 images per core
IC = OC = 256
H = W = 56
K = 3
R = 8               # output rows per matmul group
G = H // R          # 7 row groups
NCH = IC // 128     # 2 ic chunks
OCH = OC // 128     # 2 oc chunks
SLAB = 14           # x load/cast row-slab height
NSLAB = H // SLAB   # 4 slabs

# Results of the last run_bass_kernel_spmd call (exec_time_ns etc.) for
# introspection by test harnesses; not used for grading.
LAST_RESULTS = None

_CACHED_NC = None


def _build_nc() -> bass.Bass:
    nc = bacc.Bacc()
    x = nc.dram_tensor("x", [B_PER, IC, H, W], mybir.dt.float32,
                       kind="ExternalInput")
    wt = nc.dram_tensor("weight_t", [IC, K, K, OC], mybir.dt.float32,
                        kind="ExternalInput")
    bs = nc.dram_tensor("bias", [OC], mybir.dt.float32, kind="ExternalInput")
    out = nc.dram_tensor("out", [B_PER, OC, H, W], mybir.dt.float32,
                         kind="ExternalOutput")

    with tile.TileContext(nc) as tc:
        with (
            tc.tile_pool(name="const", bufs=1) as const_pool,
            tc.tile_pool(name="wprep", bufs=2) as wprep_pool,
            tc.tile_pool(name="xs", bufs=6) as xs_pool,
            tc.tile_pool(name="xp", bufs=2) as xp_pool,
            tc.tile_pool(name="osb", bufs=4) as out_pool,
            tc.tile_pool(name="psum", bufs=2, space="PSUM") as psum_pool,
        ):
            # ---- weight prep: sign-binarize into lhsT tap tiles ----
            # w_taps[ic_part, c, o, t, oc] : lhsT for (ic chunk c, oc chunk o, tap t)
            w_taps = const_pool.tile([128, NCH, OCH, K * K, 128],
                                     mybir.dt.bfloat16, tag="wtaps")

            def prep_weights(o, first=False):
                for c in range(NCH):
                    # host-permuted weight [ic, ky, kx, oc] makes both the
                    # sign read and the lhsT write fully contiguous
                    wf = wprep_pool.tile([128, K * K, 128], mybir.dt.float32,
                                         tag="wf", bufs=2, name=f"wf_{o}_{c}")
                    wsrc = wt[c * 128:(c + 1) * 128, :, :,
                              o * 128:(o + 1) * 128].rearrange(
                                  "i a b o -> i (a b) o")
                    if first and c == 0:
                        # the very first matmul waits on this: 2-queue split
                        nc.sync.dma_start(wf[:, :4, :], wsrc[:, :4, :])
                        nc.sync.dma_start(wf[:, 4:, :], wsrc[:, 4:, :])
                    else:
                        nc.sync.dma_start(wf, wsrc)
                    nc.scalar.sign(w_taps[:, c, o, :, :], wf)

            prep_weights(0, first=True)

            def load_slab(n, xpd, s, split=1):
                for c in range(NCH):
                    xs = xs_pool.tile([128, SLAB, W], mybir.dt.float32,
                                      tag="xs", name=f"xs_{n}_{s}_{c}")
                    rows = SLAB // split
                    # c==1 of the first slab posts from the scalar-engine
                    # HWDGE so both sequencers post in parallel
                    eng = nc.scalar if (split > 1 and c == 1) else nc.sync
                    for j in range(split):
                        eng.dma_start(
                            xs[:, j * rows:(j + 1) * rows, :],
                            x[n, c * 128:(c + 1) * 128,
                              s * SLAB + j * rows:s * SLAB + (j + 1) * rows,
                              :])
                    nc.vector.tensor_copy(
                        out=xpd[:, c, 1 + s * SLAB:1 + (s + 1) * SLAB,
                                1:W + 1],
                        in_=xs)

            # ---- per-image conv ----
            for n in range(B_PER):
                # zero-padded bf16 image [128, c, 58, 58]
                xpd = xp_pool.tile([128, NCH, H + 2, W + 2], mybir.dt.bfloat16,
                                   tag="xpd", name=f"xpd_{n}")
                for c in range(NCH):
                    nc.any.memset(xpd[:, c, 0, :], 0.0)
                    nc.any.memset(xpd[:, c, H + 1, :], 0.0)
                    nc.any.memset(xpd[:, c, 1:H + 1, 0], 0.0)
                    nc.any.memset(xpd[:, c, 1:H + 1, W + 1], 0.0)
                # load + cast in row slabs so matmuls can start early
                for s in range(NSLAB):
                    load_slab(n, xpd, s, split=2 if (n == 0 and s == 0) else 1)
                    if n == 0 and s == 0:
                        bias_sb = const_pool.tile([128, OCH],
                                                  mybir.dt.float32, tag="bias")
                        nc.sync.dma_start(
                            bias_sb, bs.rearrange("(a p) -> p a", p=128))
                        prep_weights(1)

                for o in range(OCH):
                    for g in range(G):
                        ps = psum_pool.tile([128, R, W], mybir.dt.float32,
                                            tag="acc", bufs=8)
                        for c in range(NCH):
                            for ky in range(K):
                                for kx in range(K):
                                    t = ky * K + kx
                                    nc.tensor.matmul(
                                        ps,
                                        w_taps[:, c, o, t, :],
                                        xpd[:, c, g * R + ky:g * R + ky + R,
                                            kx:kx + W],
                                        start=(c == 0 and t == 0),
                                        stop=(c == NCH - 1 and t == K * K - 1),
                                    )
                        osb = out_pool.tile([128, R, W], mybir.dt.float32,
                                            tag="osb")
                        nc.vector.tensor_tensor(
                            osb, ps,
                            bias_sb[:, o:o + 1, None].to_broadcast((128, R, W)),
                            mybir.AluOpType.add)
                        nc.sync.dma_start(
                            out[n, o * 128:(o + 1) * 128,
                                g * R:(g + 1) * R, :],
                            osb)
    nc.finalize()
    return nc


def kernel(x: np.ndarray, weight: np.ndarray, bias: np.ndarray) -> np.ndarray:
    global LAST_RESULTS, _CACHED_NC
    assert x.shape == (B, IC, H, W)
    if _CACHED_NC is None:
        _CACHED_NC = _build_nc()
    nc = _CACHED_NC

    # pure layout transform: [OC, IC, 3, 3] -> [IC, 3, 3, OC]
    weight_t = np.ascontiguousarray(
        np.asarray(weight, dtype=np.float32).transpose(1, 2, 3, 0))
    bias = np.ascontiguousarray(bias, dtype=np.float32)
    in_maps = [
        {
            "x": np.ascontiguousarray(x[i * B_PER:(i + 1) * B_PER],
                                      dtype=np.float32),
            "weight_t": weight_t,
            "bias": bias,
        }
        for i in range(N_CORES)
    ]
    res = run_bass_kernel_spmd(nc, in_maps, core_ids=list(range(N_CORES)))
    LAST_RESULTS = res
    return np.concatenate([res.results[i]["out"] for i in range(N_CORES)],
                          axis=0)
